# revision 3
# baseline (speedup 1.0000x reference)
"""GATv2 message-passing kernel for 8 Trainium2 NeuronCores (Bass/Tile), v2.

Strategy: host sorts edges by receiver and packs whole receiver-groups into
128-edge tiles, 16 tiles (=2048 edge slots) per "block" of <=128 distinct
receiver nodes. Blocks are dealt to 8 cores (84 blocks/core). The host also
pre-gathers per-edge sender/receiver node features (fp16, feature-major), so
the kernel is a pure dense streaming kernel: no indirect DMA, no node table.
Per block each core:
  1. 16 PE matmuls project sender features -> PSUM; DVE adds the bias ->
     gs (send projections, f32 SBUF).
  2. x = gs + recv-proj + ef@We + biases accumulated in a second PSUM
     tile (identity matmuls inject gs; receiver/edge projections
     accumulate; biases folded into the edge ones-row).
  3. ACT exp/square + DVE rational form computes mish exactly; logits +
     exp -> per-edge head weights (max-free softmax, bounded logits).
  4. a DVE is_equal builds the block-local selection matrix S
     [128e x 128slot]; 16 PE matmuls S^T @ [w*gs; w] accumulate
     numerator/denominator per node slot; divide; write [128, 64] rows.
No scatters, no collectives: every receiver's full edge set lives in
exactly one block on one core. Host scatters block rows to node ids.
"""
import sys
import os

sys.path.insert(0, "/opt/trn_rl_repo")
import numpy as np
import ml_dtypes
import concourse.bass as bass
import concourse.mybir as mybir
import concourse.tile as tile
import concourse.tile as tile_mod
from concourse.vector_clock import ScopedClock

# ---------------------------------------------------------------------------
# Environment workarounds (inlined so kernel.py is self-contained):
# 1. This walrus build only accepts ~1 sem-wait per TPB_CTRL instruction but
#    Tile piles every outstanding sem wait onto one SP drain at context exit.
#    Patch the drain to spread waits over nop carriers, and post-process all
#    instructions the same way.
# 2. Register the NEFF-emitting lowering for the 'axon' platform so 8-core
#    shard_map programs hit hardware instead of the CPU MultiCoreSim fallback.
# ---------------------------------------------------------------------------
try:
    from jax.interpreters import mlir as _mlir
    from concourse.bass2jax import (
        _bass_exec_p as _bep,
        _bass_exec_neuron_lowering as _benl,
        _partition_id_p as _pip,
        _partition_id_lowering as _pil,
    )

    _mlir.register_lowering(_bep, _benl, platform="axon")
    _mlir.register_lowering(_pip, _pil, platform="axon")
except Exception:  # pragma: no cover
    pass

_N_CARRIERS = 24


def _patched_drain_and_barrier(self, tick_clock, wait_clock):
    nc = self.nc
    nops = [nc.sync.nop(nofuse=True) for _ in range(_N_CARRIERS)]
    drain_inst = nc.sync.drain()
    wait_clock.add_sem_waits(
        drain_inst.ins, ScopedClock({None: tick_clock.global_clock}))
    waits = list(drain_inst.ins.sync_info.on_wait or [])
    if len(waits) > 1:
        assert len(waits) - 1 <= _N_CARRIERS
        drain_inst.ins.sync_info.on_wait = waits[:1]
        for nop, w in zip(nops, waits[1:]):
            si = nop.ins.sync_info
            if si is None:
                nop.ins.sync_info = mybir.SyncInfo(on_wait=[w], on_update=[])
            else:
                si.on_wait = [w]
    nc.all_engine_barrier()
    assert self.sems is not None
    popped = nc._tile_sem_poison_stack.pop()
    assert popped is self._sem_poison
    nc.clear_and_free_semaphores(list(self.sems.allocated().values()))
    nc.all_engine_barrier()


tile_mod.TileContext._drain_and_barrier = _patched_drain_and_barrier


def _split_excess_waits(nc, max_waits=1):
    for bbname, body in nc.bb_map.items():
        bb = body.bb
        insts = list(bb.instructions)
        out = []
        changed = False
        for ins in insts:
            si = ins.sync_info
            waits = list(si.on_wait) if si and si.on_wait else []
            if len(waits) > max_waits:
                keep = waits[:max_waits - 1] + [waits[-1]]
                extra = waits[max_waits - 1:-1]
                for w in extra:
                    nop = mybir.InstNoOp(
                        name=nc.get_next_instruction_name(), ins=[], outs=[])
                    nop.engine = ins.engine
                    nop.sync_info = mybir.SyncInfo(on_wait=[w], on_update=[])
                    nc.register_instruction(nop, overwrite=True)
                    out.append(nop)
                ins.sync_info.on_wait = keep
                changed = True
            out.append(ins)
        if changed:
            bb.instructions = out


F32 = mybir.dt.float32
BF16 = mybir.dt.bfloat16
F16 = mybir.dt.float16
I32 = mybir.dt.int32
BF_NP = np.dtype(ml_dtypes.bfloat16)

N_NODES = 50000
N_EDGES = 1200000
IN_DIM = 128
EDGE_DIM = 64
EMBED = 64
HEADS = 8
PAY = EMBED + HEADS  # 72

N_CORES = 8
G = 16               # tiles per block (= per super iteration)
TILE = 128
EPB = G * TILE       # 2048 edge slots per block
NSUP = 84            # blocks per core
NPAD = 50176         # node table rows (392*128)
PADNODE = 50100      # gather row for pad edges (projection of zero features)
PADLOC = 200.0       # block-local slot for pad edges: matches no slot


def _ap3(ap, mid_n):
    """[128, D] AP -> [128, mid_n(step0), D] broadcast view."""
    return bass.AP(ap.tensor, ap.offset, [ap.ap[0], [0, mid_n]] + list(ap.ap[1:]))


def _inner_b(ap, n):
    """Append a step-0 innermost free dim of size n (broadcast view)."""
    return bass.AP(ap.tensor, ap.offset, list(ap.ap) + [[0, n]])


def build_nc(nsup=NSUP, npad=NPAD):
    slots = nsup * EPB
    nt = nsup * G
    nc = bass.Bass()

    nfS_e = nc.declare_dram_parameter("nfgS", [IN_DIM, slots], F16, isOutput=False)
    nfR_e = nc.declare_dram_parameter("nfgR", [IN_DIM, slots], F16, isOutput=False)
    W_e = nc.declare_dram_parameter("W", [IN_DIM, EMBED], F16, isOutput=False)
    Wb_e = nc.declare_dram_parameter("Wbr", [1, G * EMBED], F16, isOutput=False)
    eftT_e = nc.declare_dram_parameter("eftT", [EDGE_DIM + 1, slots], F16, isOutput=False)
    We65_e = nc.declare_dram_parameter("We65", [EDGE_DIM + 1, EMBED], F16, isOutput=False)
    rl_e = nc.declare_dram_parameter("rloc", [128, nt], BF16, isOutput=False)
    a16_e = nc.declare_dram_parameter("a16", [128, G * EMBED], F16, isOutput=False)
    iota_e = nc.declare_dram_parameter("iotaf", [128, 128], BF16, isOutput=False)
    out_e = nc.declare_dram_parameter("out_shard", [nsup * 128, EMBED], F32, isOutput=True)

    with tile.TileContext(nc) as tc:
        with (
            nc.allow_low_precision(reason="output tolerance 2e-2; bf16 is ample"),
            tc.tile_pool(name="const", bufs=1) as cpool,
            tc.tile_pool(name="edgea", bufs=3) as apool,
            tc.tile_pool(name="edgeb", bufs=4) as bpool,
            tc.tile_pool(name="small", bufs=2) as spool,
            tc.tile_pool(name="ps_g", bufs=3, space="PSUM") as ps_g,
            tc.tile_pool(name="ps_b", bufs=2, space="PSUM") as ps_b,
        ):
            # ---- constants
            W_t = cpool.tile([IN_DIM, EMBED], F16)
            nc.sync.dma_start(out=W_t[:], in_=W_e[:])
            Wb_t = cpool.tile([1, G * EMBED], F16)
            nc.sync.dma_start(out=Wb_t[:], in_=Wb_e[:])
            ones1 = cpool.tile([1, 128], F16)
            nc.gpsimd.memset(ones1[:], 1.0)
            We65_t = cpool.tile([EDGE_DIM + 1, EMBED], F16)
            nc.sync.dma_start(out=We65_t[:], in_=We65_e[:])
            a16_t = cpool.tile([128, G * EMBED], F16)
            nc.sync.dma_start(out=a16_t[:], in_=a16_e[:])
            iota_t = cpool.tile([128, 128], BF16)
            nc.sync.dma_start(out=iota_t[:], in_=iota_e[:])

            # ---- phase 2: blocks, software-pipelined (loads of s+LOOKAHEAD
            # are emitted before compute of s so gathers stay ahead).
            LOOKAHEAD = 2

            def stage_a(s):
                rL = apool.tile([128, G], BF16, tag="rloc")
                nc.sync.dma_start(out=rL[:], in_=rl_e[:, s * G:(s + 1) * G])
                ef_t = apool.tile([EDGE_DIM + 1, EPB], F16, tag="eft")
                nc.sync.dma_start(
                    out=ef_t[:], in_=eftT_e[:, s * EPB:(s + 1) * EPB])
                fS = apool.tile([IN_DIM, EPB], F16, tag="fS")
                nc.sync.dma_start(
                    out=fS[:], in_=nfS_e[:, s * EPB:(s + 1) * EPB])
                fR = apool.tile([IN_DIM, EPB], F16, tag="fR")
                nc.sync.dma_start(
                    out=fR[:], in_=nfR_e[:, s * EPB:(s + 1) * EPB])
                return rL, ef_t, fS, fR

            def stage_b1(s, ctx):
                rL, ef_t, fS, fR = ctx
                # sender projections + bias -> PSUM; gs = f16 copy (ACT)
                pg = ps_g.tile([128, G, EMBED], F32, space="PSUM", tag="pg")
                pgf = pg[:].rearrange("p c d -> p (c d)")
                H = G * EMBED // 2
                for g in range(G):
                    nc.tensor.matmul(
                        out=pg[:, g, :], lhsT=fS[:, g * 128:(g + 1) * 128],
                        rhs=W_t[:], start=(g == 0 or g == 8), stop=False,
                        skip_group_check=True)
                nc.tensor.matmul(out=pgf[:, :H], lhsT=ones1[:], rhs=Wb_t[:, :H],
                                 start=False, stop=False, skip_group_check=True)
                nc.tensor.matmul(out=pgf[:, H:], lhsT=ones1[:], rhs=Wb_t[:, H:],
                                 start=False, stop=False, skip_group_check=True)
                gs = bpool.tile([128, G, EMBED], F16, tag="gs")
                nc.scalar.activation(
                    gs[:].rearrange("p c d -> p (c d)"), pgf,
                    mybir.ActivationFunctionType.Copy)
                return ctx + (pg, gs)

            def stage_b2(s, ctx):
                rL, ef_t, fS, fR, pg, gs = ctx
                xp = pg
                xf = xp[:].rearrange("p c d -> p (c d)")
                # x = send(+Wb) + recv + ef@We + (Web + Wb), accumulated in
                # the same PSUM tile after gs was extracted
                for g in range(G):
                    nc.tensor.matmul(
                        out=xp[:, g, :], lhsT=fR[:, g * 128:(g + 1) * 128],
                        rhs=W_t[:], start=False, stop=False,
                        skip_group_check=True)
                for g in range(G):
                    nc.tensor.matmul(
                        out=xp[:, g, :], lhsT=ef_t[:, g * 128:(g + 1) * 128],
                        rhs=We65_t[:], start=False,
                        stop=(g == 7 or g == G - 1), skip_group_check=True)
                # stage x to SBUF f16 (cheap ACT reads; DVE then runs at
                # 16-bit rate), then mish(x) = x*(v^2-1)/(v^2+1), v=e^x+1
                # (exact; f16 saturation at large x gives the t -> 1 limit)
                xs = bpool.tile([128, G * EMBED], F16, tag="xs")
                nc.scalar.activation(xs[:], xf,
                                     mybir.ActivationFunctionType.Copy)
                u_t = bpool.tile([128, G * EMBED], F16, tag="mu")
                nc.scalar.activation(u_t[:], xs[:],
                                     mybir.ActivationFunctionType.Exp)
                q_t = bpool.tile([128, G * EMBED], F16, tag="mq")
                nc.scalar.activation(q_t[:], u_t[:],
                                     mybir.ActivationFunctionType.Square,
                                     bias=1.0)
                c_t = bpool.tile([128, G * EMBED], F16, tag="mc")
                nc.vector.tensor_scalar(
                    out=c_t[:], in0=q_t[:], scalar1=60000.0, scalar2=1.0,
                    op0=mybir.AluOpType.min, op1=mybir.AluOpType.add)
                d_t = bpool.tile([128, G * EMBED], F16, tag="md")
                nc.vector.reciprocal(d_t[:], c_t[:])
                t_t = bpool.tile([128, G * EMBED], F16, tag="mt")
                nc.vector.tensor_scalar(
                    out=t_t[:], in0=d_t[:], scalar1=-2.0, scalar2=1.0,
                    op0=mybir.AluOpType.mult, op1=mybir.AluOpType.add)
                xm_t = bpool.tile([128, G * EMBED], F16, tag="xm")
                nc.vector.tensor_mul(xm_t[:], xs[:], t_t[:])
                # logits per head (mul on DVE, segment reduce on idle Pool)
                lgm = bpool.tile([128, G * EMBED], F16, tag="lgm")
                nc.vector.tensor_mul(lgm[:], xm_t[:], a16_t[:])
                lg = bpool.tile([128, G * HEADS], F32, tag="lg")
                nc.vector.tensor_reduce(
                    lg[:].rearrange("p (g o) -> p g o", o=1),
                    lgm[:].rearrange("p (g i) -> p g i", i=8),
                    axis=mybir.AxisListType.X, op=mybir.AluOpType.add)
                pay = bpool.tile([128, G, PAY], BF16, tag="pay")
                nc.scalar.activation(
                    pay[:, :, EMBED:], lg[:].rearrange("p (c h) -> p c h", h=HEADS),
                    mybir.ActivationFunctionType.Exp)
                nc.vector.tensor_mul(
                    pay[:, :, :EMBED].rearrange("p c (h o) -> p c h o", o=8),
                    gs[:].rearrange("p c (h o) -> p c h o", o=8),
                    _inner_b(pay[:, :, EMBED:], 8))
                # block-local selection matrix + segment sums
                S_t = bpool.tile([128, G, 128], BF16, tag="S")
                nc.vector.tensor_tensor(
                    out=S_t[:], in0=_inner_b(rL[:], 128),
                    in1=_ap3(iota_t[:], G), op=mybir.AluOpType.is_equal)
                return pay, S_t

            def stage_b2c(s, ctx):
                pay, S_t = ctx
                blk = ps_b.tile([128, PAY], F32, space="PSUM", tag="blk")
                for g in range(G):
                    nc.tensor.matmul(
                        out=blk[:], lhsT=S_t[:, g, :], rhs=pay[:, g, :],
                        start=(g == 0), stop=(g == G - 1))
                den = spool.tile([128, HEADS], F32, tag="den")
                nc.vector.tensor_scalar_add(den[:], blk[:, EMBED:], 1e-30)
                rec = spool.tile([128, HEADS], F32, tag="rec")
                nc.vector.reciprocal(rec[:], den[:])
                ot = spool.tile([128, EMBED], F32, tag="ot")
                nc.vector.tensor_mul(
                    ot[:].rearrange("p (h o) -> p h o", o=8),
                    blk[:, :EMBED].rearrange("p (h o) -> p h o", o=8),
                    _inner_b(rec[:], 8))
                nc.sync.dma_start(out=out_e[s * 128:(s + 1) * 128, :], in_=ot[:])

            pend_a, pend_b, pend_c = {}, {}, {}
            for s in range(nsup):
                pend_a[s] = stage_a(s)
                if s - 1 >= 0:
                    pend_b[s - 1] = stage_b1(s - 1, pend_a.pop(s - 1))
                if s - 2 >= 0:
                    pend_c[s - 2] = stage_b2(s - 2, pend_b.pop(s - 2))
                if s - 3 >= 0:
                    stage_b2c(s - 3, pend_c.pop(s - 3))
            for s in sorted(pend_a):
                pend_b[s] = stage_b1(s, pend_a.pop(s))
            for s in sorted(pend_b):
                pend_c[s] = stage_b2(s, pend_b.pop(s))
            for s in sorted(pend_c):
                stage_b2c(s, pend_c.pop(s))

    _split_excess_waits(nc)
    return nc


def pack_edges(receivers, n_nodes, nsup=NSUP, n_cores=N_CORES):
    """Sort edges by receiver, pack whole receiver-groups into 128-edge
    tiles, 16 tiles per block (<=128 nodes per block). Returns:
      edge_slot  [E] -> global slot id in [0, n_cores*nsup*EPB)
      node_map   [n_cores*nsup, 128] -> global node id or -1
    Slot layout: core-major, then block, then tile, then lane.
    """
    order = np.argsort(receivers, kind="stable")
    deg = np.bincount(receivers, minlength=n_nodes)
    present = np.flatnonzero(deg)
    degs = deg[present]
    assert degs.max() <= TILE, "receiver degree exceeds one tile"

    n_blocks_cap = n_cores * nsup
    node_map = np.full((n_blocks_cap, 128), -1, np.int64)
    node_base = np.empty(len(present), np.int64)  # slot of group start

    blk = 0
    cur_tiles = 0   # completed tiles in current block
    tile_fill = 0
    block_nodes = 0
    for i in range(len(present)):
        d = degs[i]
        if tile_fill + d > TILE:
            cur_tiles += 1
            tile_fill = 0
        if cur_tiles == G or block_nodes == 128:
            blk += 1
            cur_tiles = 0
            tile_fill = 0
            block_nodes = 0
        assert blk < n_blocks_cap, "packing overflow: raise NSUP"
        node_map[blk, block_nodes] = present[i]
        node_base[i] = (blk * G + cur_tiles) * TILE + tile_fill
        tile_fill += d
        block_nodes += 1

    # per-edge slots (order-sorted edges get consecutive lanes per group)
    gstart = np.zeros(len(present), np.int64)
    np.cumsum(degs[:-1], out=gstart[1:])
    edge_slot_sorted = np.repeat(node_base, degs) + (
        np.arange(len(order)) - np.repeat(gstart, degs))
    edge_slot = np.empty(len(order), np.int64)
    edge_slot[order] = edge_slot_sorted

    # block-local node slot for each node (via node_map layout)
    slots = np.tile(np.arange(128), n_blocks_cap)
    flat = node_map.ravel()
    valid = flat >= 0
    slot_of_node = np.full(n_nodes, -1, np.int64)
    slot_of_node[flat[valid]] = slots[valid]
    return edge_slot, node_map, slot_of_node, blk + 1


def host_prep(node_features, edge_features, senders, receivers,
              W_kernel, W_bias, We_kernel, We_bias, a,
              n_cores=N_CORES, nsup=NSUP, npad=NPAD, n_nodes=N_NODES):
    """Pure layout transforms -> per-core input maps + assembly info."""
    slots_pc = nsup * EPB
    nt = nsup * G
    total_slots = n_cores * slots_pc

    edge_slot, node_map, slot_of_node, n_blocks = pack_edges(
        receivers, n_nodes, nsup=nsup, n_cores=n_cores)

    padnode = n_nodes  # zero-feature column; its S column is all-zero anyway
    s_all = np.full(total_slots, padnode, np.int64)
    r_all = np.full(total_slots, padnode, np.int64)
    rl_all = np.full(total_slots, PADLOC, np.float32)
    ef_all = np.zeros((total_slots, EDGE_DIM), np.float32)
    s_all[edge_slot] = senders
    r_all[edge_slot] = receivers
    rl_all[edge_slot] = slot_of_node[receivers]
    ef_all[edge_slot] = edge_features

    # feature-major fp16 node features with a trailing zero pad column
    nfT16 = np.zeros((IN_DIM, n_nodes + 1), np.float16)
    nfT16[:, :n_nodes] = np.asarray(node_features, np.float32).T

    Wbr = np.tile(np.asarray(W_bias, np.float32)[None, :],
                  (1, G)).astype(np.float16)
    a16 = np.tile(np.asarray(a, np.float32).reshape(-1)[None, :],
                  (128, G)).astype(np.float16)
    # ones-row bias carries We_bias + W_bias (recv proj bias; send bias is
    # added separately so the payload sees biased send projections)
    We65 = np.concatenate(
        [np.asarray(We_kernel, np.float32),
         (np.asarray(We_bias, np.float32)
          + np.asarray(W_bias, np.float32))[None, :]], axis=0).astype(np.float16)
    iotaf = np.tile(np.arange(128, dtype=np.float32)[None, :], (128, 1)).astype(BF_NP)
    W_16 = np.asarray(W_kernel, np.float32).astype(np.float16)

    in_maps = []
    for c in range(n_cores):
        lo = c * slots_pc
        hi = lo + slots_pc
        ef_c = ef_all[lo:hi]  # [slots_pc, 64]
        eftT = np.empty((EDGE_DIM + 1, slots_pc), np.float16)
        eftT[:EDGE_DIM] = ef_c.T
        eftT[EDGE_DIM] = 1.0
        in_maps.append({
            "nfgS": nfT16[:, s_all[lo:hi]],
            "nfgR": nfT16[:, r_all[lo:hi]],
            "W": W_16,
            "Wbr": Wbr,
            "eftT": eftT,
            "We65": We65,
            "rloc": np.ascontiguousarray(
                rl_all[lo:hi].reshape(nt, TILE).T).astype(BF_NP),
            "a16": a16,
            "iotaf": iotaf,
        })
    return in_maps, node_map


def _build_runner(nc, n_cores):
    """Jitted 8-core SPMD executor via the axon PJRT tunnel (shard_map)."""
    import time
    import jax
    from jax.sharding import Mesh, PartitionSpec
    from jax.experimental.shard_map import shard_map
    from concourse import bass2jax
    from concourse.bass2jax import _bass_exec_p, install_neuronx_cc_hook

    install_neuronx_cc_hook()
    partition_name = nc.partition_id_tensor.name if nc.partition_id_tensor else None
    in_names, out_names, out_avals, zero_outs = [], [], [], []
    for alloc in nc.m.functions[0].allocations:
        if not isinstance(alloc, mybir.MemoryLocationSet):
            continue
        name = alloc.memorylocations[0].name
        if alloc.kind == "ExternalInput":
            if name != partition_name:
                in_names.append(name)
        elif alloc.kind == "ExternalOutput":
            out_names.append(name)
            shape = tuple(alloc.tensor_shape)
            dtype = mybir.dt.np(alloc.dtype)
            out_avals.append(jax.core.ShapedArray(shape, dtype))
            zero_outs.append(np.zeros(shape, dtype))
    n_params = len(in_names)
    n_outs = len(out_avals)
    all_in_names = list(in_names) + list(out_names)
    if partition_name is not None:
        all_in_names.append(partition_name)

    def _body(*args):
        operands = list(args)
        if partition_name is not None:
            operands.append(bass2jax.partition_id_tensor())
        return tuple(_bass_exec_p.bind(
            *operands,
            out_avals=tuple(out_avals),
            in_names=tuple(all_in_names),
            out_names=tuple(out_names),
            lowering_input_output_aliases=(),
            sim_require_finite=True,
            sim_require_nnan=True,
            nc=nc,
        ))

    donate = tuple(range(n_params, n_params + n_outs))
    devices = jax.devices()[:n_cores]
    mesh = Mesh(np.asarray(devices), ("core",))
    in_specs = (PartitionSpec("core"),) * (n_params + n_outs)
    out_specs = (PartitionSpec("core"),) * len(out_names)
    jfn = jax.jit(
        shard_map(_body, mesh=mesh, in_specs=in_specs, out_specs=out_specs,
                  check_rep=False),
        donate_argnums=donate, keep_unused=True)

    def fn(in_maps):
        concat_in = [
            np.concatenate([np.asarray(in_maps[c][n]) for c in range(n_cores)], 0)
            for n in in_names
        ]
        concat_zeros = [np.zeros((n_cores * z.shape[0], *z.shape[1:]), z.dtype)
                        for z in zero_outs]
        t0 = time.perf_counter()
        out_arrs = jfn(*concat_in, *concat_zeros)
        out_arrs = [np.asarray(o) for o in out_arrs]
        dt = time.perf_counter() - t0
        return [
            {n: out_arrs[i].reshape(n_cores, *out_avals[i].shape)[c]
             for i, n in enumerate(out_names)}
            for c in range(n_cores)
        ], dt

    return fn


_CACHE = {}


def kernel(node_features, edge_features, global_features, senders, receivers,
           W_kernel, W_bias, We_kernel, We_bias, a):
    node_features = np.asarray(node_features, np.float32)
    edge_features = np.asarray(edge_features, np.float32)
    senders = np.asarray(senders, np.int32)
    receivers = np.asarray(receivers, np.int32)
    in_maps, node_map = host_prep(
        node_features, edge_features, senders, receivers,
        W_kernel, W_bias, We_kernel, We_bias, a)
    if "fn" not in _CACHE:
        nc = build_nc()
        _CACHE["fn"] = _build_runner(nc, N_CORES)
    res, dt = _CACHE["fn"](in_maps)
    _CACHE["last_dt"] = dt
    rows = np.concatenate(
        [r["out_shard"].reshape(NSUP * 128, EMBED) for r in res], axis=0)
    full = np.zeros((N_NODES, EMBED), np.float32)
    flat_map = node_map.reshape(-1)
    valid = flat_map >= 0
    full[flat_map[valid]] = rows[valid]
    return full
